# Initial kernel scaffold
#
"""2-layer GAT on 8 Trainium2 NeuronCores (Bass/Tile).

Strategy (graph partition by dst):
- Nodes sorted by in-degree, snake-dealt across 8 cores (6250 -> padded 6272
  per core), then tiled 128/tile (49 tiles). Partition j of tile t owns one
  dst node; its incoming edges occupy "slots" (chunk c, partition j).
- Per-core node table in HBM: row = [f(256) | el(8)] f32 built by the
  projection matmul x @ [W1|W1@al1|W1@ar1]; per-edge rows fetched by
  indirect DMA gather (128 rows per chunk).
- alpha = exp(leaky_relu(el[src]+er[dst])) with no max-subtraction (logits
  are small; softmax is shift-invariant). alpha written into the gathered
  tile's el column; one identity-matmul per chunk accumulates
  [sum(alpha*f) | sum(alpha)] in PSUM; divide, +b1, ELU.
- Layer-2 projection h1 @ [W2|wl2|wr2] per own tile; slices returned to the
  host, which assembles the full layer-2 table for launch 2 (same edge
  grids, 34-float rows).
- Padding edge slots point at a sentinel row (f=0, el=-300 -> alpha ~= 0).
"""
import sys

sys.path.insert(0, "/opt/trn_rl_repo")

import numpy as np

import concourse.bass as bass
import concourse.bacc as bacc
import concourse.tile as tile
from concourse import mybir
from concourse.bass_utils import run_bass_kernel_spmd

N = 50000
E = 800000
P = 128
NCORES = 8
TILES = 49                       # tiles per core
NPC = TILES * P                  # 6272 nodes per core
NPAD = NCORES * NPC              # 50176
GBLOCKS = NPAD // P              # 392 projection blocks
SPLIT_ROW = 25088                # sentinel A position (core 4 boundary)
NTAB = NPAD + 2                  # 50178 rows (two sentinels)
SENT_A = SPLIT_ROW               # sentinel rows in table space
SENT_B = NTAB - 1
ROW1 = 264                       # [f 256 | el 8]
ROW2 = 34                        # [f2 32 | el2 1 | er2 1]
H1, D1 = 8, 32
NEG_SLOPE = 0.2
SENT_EL = -300.0
F32 = mybir.dt.float32
I32 = mybir.dt.int32


def _new_row(r):
    """table row of node position r (insert sentinel at SPLIT_ROW)."""
    return r + (r >= SPLIT_ROW)


# ----------------------------------------------------------------------------
# host preprocessing
# ----------------------------------------------------------------------------

def _prep(src, dst):
    deg = np.bincount(dst, minlength=N)
    order = np.argsort(-deg, kind="stable")
    pat = np.concatenate([np.arange(NCORES), np.arange(NCORES - 1, -1, -1)])
    core_of_pos = pat[np.arange(N) % (2 * NCORES)]
    newid = np.empty(N, np.int64)
    for c in range(NCORES):
        nodes_c = order[core_of_pos == c]
        newid[nodes_c] = c * NPC + np.arange(len(nodes_c))

    nd = newid[dst]
    ns = newid[src]
    core_e = nd // NPC
    t_e = (nd % NPC) // P

    o = np.argsort(nd, kind="stable")
    nd_s, ns_s = nd[o], ns[o]
    first = np.searchsorted(nd_s, np.arange(NPAD), side="left")
    k_s = np.arange(E) - first[nd_s]

    degn = np.bincount(nd, minlength=NPAD).reshape(NCORES, TILES, P)
    T = degn.max(axis=(0, 2)).clip(min=1).astype(np.int64)   # [TILES]
    offs = np.concatenate([[0], np.cumsum(T)])
    TS = int(offs[-1])

    # per-core row order: own 49 blocks first, then the rest
    # rowpos_c[global_block] = position in core c's table
    blockpos = np.empty((NCORES, GBLOCKS), np.int64)
    xt_order = np.empty((NCORES, GBLOCKS), np.int64)
    for c in range(NCORES):
        own = np.arange(c * TILES, (c + 1) * TILES)
        rest = np.concatenate(
            [np.arange(0, c * TILES), np.arange((c + 1) * TILES, GBLOCKS)]
        )
        bo = np.concatenate([own, rest])        # xt block order for core c
        xt_order[c] = bo
        blockpos[c][bo] = np.arange(GBLOCKS)

    # gather indices: per-core table rows of edge srcs
    idxs = np.full((NCORES, P, TS), SENT_B, np.int32)
    c_s = core_e[o]
    t_s = t_e[o]
    j_s = (nd_s % P).astype(np.int64)
    slot_s = offs[t_s] + k_s
    g_src = ns_s // P
    rowpos = blockpos[c_s, g_src] * P + (ns_s % P)
    idxs[c_s, j_s, slot_s] = _new_row(rowpos).astype(np.int32)

    return {
        "newid": newid,
        "T": T,
        "offs": offs,
        "TS": TS,
        "idxs": idxs,
        "xt_order": xt_order,
        "blockpos": blockpos,
    }


# ----------------------------------------------------------------------------
# launch 1: projection + layer-1 edges + layer-2 projection
# ----------------------------------------------------------------------------

def _build_launch1(T):
    TS = int(T.sum())
    nc = bacc.Bacc("TRN2", target_bir_lowering=False, debug=False,
                   num_devices=NCORES)
    xt = nc.dram_tensor("xt", [GBLOCKS, P, P], F32, kind="ExternalInput")
    w1aug = nc.dram_tensor("w1aug", [P, 272], F32, kind="ExternalInput")
    w2aug = nc.dram_tensor("w2aug", [256, ROW2], F32, kind="ExternalInput")
    identin = nc.dram_tensor("identin", [P, P], F32, kind="ExternalInput")
    sentin = nc.dram_tensor("sentin", [1, ROW1], F32, kind="ExternalInput")
    idxin = nc.dram_tensor("idxin", [P, TS], I32, kind="ExternalInput")
    f2out = nc.dram_tensor("f2out", [NPC, ROW2], F32, kind="ExternalOutput")
    table = nc.dram_tensor("table", [NTAB, ROW1], F32, kind="Internal")

    er_sb = nc.alloc_sbuf_tensor("er_sb", [P, TILES * H1], F32).ap()
    idx_sb = nc.alloc_sbuf_tensor("idx_sb", [P, TS], I32).ap()
    ident_sb = nc.alloc_sbuf_tensor("ident_sb", [P, P], F32).ap()
    w2_sb = nc.alloc_sbuf_tensor("w2_sb", [256, ROW2], F32).ap()

    # ---- phase 1: projection, builds the full node table --------------------
    with tile.TileContext(nc) as tc:
        with (
            tc.tile_pool(name="p1sbuf", bufs=3) as pool,
            tc.tile_pool(name="p1psum", bufs=4, space="PSUM") as psum,
            tc.tile_pool(name="p1const", bufs=1) as consts,
        ):
            w1_sb = consts.tile([P, 272], F32)
            nc.sync.dma_start(out=w1_sb[:], in_=w1aug[:])
            nc.sync.dma_start(out=ident_sb, in_=identin[:])
            nc.sync.dma_start(out=w2_sb, in_=w2aug[:])
            nc.sync.dma_start(out=idx_sb, in_=idxin[:])
            sent_sb = consts.tile([1, ROW1], F32)
            nc.sync.dma_start(out=sent_sb[:], in_=sentin[:])
            nc.sync.dma_start(out=table[SENT_A:SENT_A + 1, :], in_=sent_sb[:])
            nc.sync.dma_start(out=table[SENT_B:SENT_B + 1, :], in_=sent_sb[:])
            for b in range(GBLOCKS):
                xtile = pool.tile([P, P], F32, tag="xt")
                nc.sync.dma_start(out=xtile[:], in_=xt[b])
                pp = psum.tile([P, 272], F32)
                nc.tensor.matmul(pp[:], xtile[:], w1_sb[:, :],
                                 start=True, stop=True)
                fo = pool.tile([P, ROW1], F32, tag="fo")
                nc.scalar.activation(out=fo[:], in_=pp[:, 0:ROW1],
                                     func=mybir.ActivationFunctionType.Copy)
                if b < TILES:
                    nc.vector.tensor_copy(
                        out=er_sb[:, b * H1:(b + 1) * H1], in_=pp[:, 264:272]
                    )
                r0 = _new_row(b * P)
                nc.sync.dma_start(out=table[r0:r0 + P, :], in_=fo[:])

    # ---- phase 2: layer-1 edge aggregation + layer-2 projection -------------
    offs = np.concatenate([[0], np.cumsum(T)])
    with tile.TileContext(nc) as tc:
        with (
            tc.tile_pool(name="p2sbuf", bufs=2) as pool,
            tc.tile_pool(name="p2small", bufs=3) as small,
            tc.tile_pool(name="p2psum", bufs=2, space="PSUM") as psum,
            tc.tile_pool(name="p2psumT", bufs=2, space="PSUM") as psumT,
            tc.tile_pool(name="p2psum2", bufs=2, space="PSUM") as psum2,
            tc.tile_pool(name="p2const", bufs=1) as consts,
        ):
            Tmax = int(T.max())
            half = consts.tile([P, 1], F32)
            nc.vector.memset(half[:], NEG_SLOPE)
            for t in range(TILES):
                Tt = int(T[t])
                o0 = int(offs[t])
                g = pool.tile([P, Tmax * ROW1], F32, tag="g")
                gv = g[:].rearrange("p (c f) -> p c f", f=ROW1)
                for c in range(Tt):
                    nc.gpsimd.indirect_dma_start(
                        out=gv[:, c, :],
                        out_offset=None,
                        in_=table[:],
                        in_offset=bass.IndirectOffsetOnAxis(
                            ap=idx_sb[:, o0 + c:o0 + c + 1], axis=0
                        ),
                    )
                # logits: lt = el[src] + er[dst]  [P, Tt*8]
                lt = small.tile([P, Tmax * H1], F32, tag="lt")
                el_ap = bass.AP(g.tensor, g.offset + 256,
                                [g.ap[0], [ROW1, Tt], [1, H1]])
                er_ap = bass.AP(er_sb.tensor, er_sb.offset + t * H1,
                                [er_sb.ap[0], [0, Tt], [1, H1]])
                lt_ap = bass.AP(lt.tensor, lt.offset,
                                [lt.ap[0], [H1, Tt], [1, H1]])
                nc.vector.tensor_tensor(out=lt_ap, in0=el_ap, in1=er_ap,
                                        op=mybir.AluOpType.add)
                # leaky relu: lt = max(lt, 0.2*lt)
                lt2 = small.tile([P, Tmax * H1], F32, tag="lt2")
                nc.vector.tensor_scalar_mul(lt2[:, :Tt * H1], lt[:, :Tt * H1],
                                            half[:, 0:1])
                nc.vector.tensor_tensor(out=lt[:, :Tt * H1],
                                        in0=lt[:, :Tt * H1],
                                        in1=lt2[:, :Tt * H1],
                                        op=mybir.AluOpType.max)
                # alpha = exp(lt), written into the el column of g
                al_ap = bass.AP(g.tensor, g.offset + 256,
                                [g.ap[0], [ROW1, Tt], [1, H1]])
                nc.scalar.activation(out=al_ap, in_=lt_ap,
                                     func=mybir.ActivationFunctionType.Exp)
                # msg scale: g[:, :, 0:256] *= alpha (broadcast over d)
                f_ap = bass.AP(g.tensor, g.offset,
                               [g.ap[0], [ROW1, Tt], [32, H1], [1, 32]])
                ab_ap = bass.AP(g.tensor, g.offset + 256,
                                [g.ap[0], [ROW1, Tt], [1, H1], [0, 32]])
                nc.vector.tensor_tensor(out=f_ap, in0=f_ap, in1=ab_ap,
                                        op=mybir.AluOpType.mult)
                # aggregate: acc = [sum alpha*f | sum alpha]
                acc = psum.tile([P, ROW1], F32, tag="acc")
                for c in range(Tt):
                    nc.tensor.matmul(acc[:], ident_sb, gv[:, c, :],
                                     start=(c == 0), stop=(c == Tt - 1))
                # h1 = elu(acc[:, :256] / denom + b1)   (b1 == 0)
                rec = small.tile([P, H1], F32, tag="rec")
                nc.vector.reciprocal(rec[:], acc[:, 256:ROW1])
                h1 = pool.tile([P, 256], F32, tag="h1")
                acc_f = bass.AP(acc.tensor, acc.offset,
                                [acc.ap[0], [32, H1], [1, 32]])
                rb_ap = bass.AP(rec.tensor, rec.offset,
                                [rec.ap[0], [1, H1], [0, 32]])
                h1_ap = bass.AP(h1.tensor, h1.offset,
                                [h1.ap[0], [32, H1], [1, 32]])
                nc.vector.tensor_tensor(out=h1_ap, in0=acc_f, in1=rb_ap,
                                        op=mybir.AluOpType.mult)
                # ELU: h1 = max(h1, exp(min(h1,0)) - 1)
                e1 = pool.tile([P, 256], F32, tag="e1")
                nc.vector.tensor_scalar_min(e1[:], h1[:], 0.0)
                nc.scalar.activation(out=e1[:], in_=e1[:],
                                     func=mybir.ActivationFunctionType.Exp)
                nc.vector.tensor_scalar_add(e1[:], e1[:], -1.0)
                nc.vector.tensor_tensor(out=h1[:], in0=h1[:], in1=e1[:],
                                        op=mybir.AluOpType.max)
                # layer-2 projection: f2 = h1 @ w2aug
                f2p = psum2.tile([P, ROW2], F32, tag="f2p")
                for k in range(2):
                    tp = psumT.tile([P, P], F32, tag="tp")
                    nc.tensor.transpose(out=tp[:],
                                        in_=h1[:, k * P:(k + 1) * P],
                                        identity=ident_sb)
                    h1t = small.tile([P, P], F32, tag="h1t")
                    nc.vector.tensor_copy(out=h1t[:], in_=tp[:])
                    nc.tensor.matmul(f2p[:], h1t[:], w2_sb[k * P:(k + 1) * P, :],
                                     start=(k == 0), stop=(k == 1))
                f2s = small.tile([P, ROW2], F32, tag="f2s")
                nc.scalar.activation(out=f2s[:], in_=f2p[:],
                                     func=mybir.ActivationFunctionType.Copy)
                nc.sync.dma_start(out=f2out[t * P:(t + 1) * P, :], in_=f2s[:])
    nc.compile()
    return nc


# ----------------------------------------------------------------------------
# launch 2: layer-2 edge aggregation
# ----------------------------------------------------------------------------

def _build_launch2(T):
    TS = int(T.sum())
    nc = bacc.Bacc("TRN2", target_bir_lowering=False, debug=False,
                   num_devices=NCORES)
    table2 = nc.dram_tensor("table2", [NTAB, ROW2], F32, kind="ExternalInput")
    idxin = nc.dram_tensor("idxin", [P, TS], I32, kind="ExternalInput")
    er2in = nc.dram_tensor("er2in", [P, TILES], F32, kind="ExternalInput")
    identin = nc.dram_tensor("identin", [P, P], F32, kind="ExternalInput")
    outbuf = nc.dram_tensor("outbuf", [NPC, 32], F32, kind="ExternalOutput")

    offs = np.concatenate([[0], np.cumsum(T)])
    with tile.TileContext(nc) as tc:
        with (
            tc.tile_pool(name="l2sbuf", bufs=2) as pool,
            tc.tile_pool(name="l2small", bufs=3) as small,
            tc.tile_pool(name="l2psum", bufs=3, space="PSUM") as psum,
            tc.tile_pool(name="l2const", bufs=1) as consts,
        ):
            Tmax = int(T.max())
            ident_sb = consts.tile([P, P], F32)
            nc.sync.dma_start(out=ident_sb[:], in_=identin[:])
            idx_sb = consts.tile([P, TS], I32)
            nc.sync.dma_start(out=idx_sb[:], in_=idxin[:])
            er2_sb = consts.tile([P, TILES], F32)
            nc.sync.dma_start(out=er2_sb[:], in_=er2in[:])
            half = consts.tile([P, 1], F32)
            nc.vector.memset(half[:], NEG_SLOPE)
            for t in range(TILES):
                Tt = int(T[t])
                o0 = int(offs[t])
                g = pool.tile([P, Tmax * ROW2], F32, tag="g")
                gv = g[:].rearrange("p (c f) -> p c f", f=ROW2)
                for c in range(Tt):
                    nc.gpsimd.indirect_dma_start(
                        out=gv[:, c, :],
                        out_offset=None,
                        in_=table2[:],
                        in_offset=bass.IndirectOffsetOnAxis(
                            ap=idx_sb[:, o0 + c:o0 + c + 1], axis=0
                        ),
                    )
                lt = small.tile([P, Tmax], F32, tag="lt")
                el_ap = bass.AP(g.tensor, g.offset + 32,
                                [g.ap[0], [ROW2, Tt]])
                er_ap = bass.AP(er2_sb.tensor, er2_sb.offset + t,
                                [er2_sb.ap[0], [0, Tt]])
                nc.vector.tensor_tensor(out=lt[:, :Tt], in0=el_ap, in1=er_ap,
                                        op=mybir.AluOpType.add)
                lt2 = small.tile([P, Tmax], F32, tag="lt2")
                nc.vector.tensor_scalar_mul(lt2[:, :Tt], lt[:, :Tt],
                                            half[:, 0:1])
                nc.vector.tensor_tensor(out=lt[:, :Tt], in0=lt[:, :Tt],
                                        in1=lt2[:, :Tt],
                                        op=mybir.AluOpType.max)
                al_ap = bass.AP(g.tensor, g.offset + 32,
                                [g.ap[0], [ROW2, Tt]])
                nc.scalar.activation(out=al_ap, in_=lt[:, :Tt],
                                     func=mybir.ActivationFunctionType.Exp)
                f_ap = bass.AP(g.tensor, g.offset,
                               [g.ap[0], [ROW2, Tt], [1, 32]])
                ab_ap = bass.AP(g.tensor, g.offset + 32,
                                [g.ap[0], [ROW2, Tt], [0, 32]])
                nc.vector.tensor_tensor(out=f_ap, in0=f_ap, in1=ab_ap,
                                        op=mybir.AluOpType.mult)
                acc = psum.tile([P, 33], F32, tag="acc")
                for c in range(Tt):
                    nc.tensor.matmul(acc[:], ident_sb[:], gv[:, c, 0:33],
                                     start=(c == 0), stop=(c == Tt - 1))
                rec = small.tile([P, 1], F32, tag="rec")
                nc.vector.reciprocal(rec[:], acc[:, 32:33])
                o2 = small.tile([P, 32], F32, tag="o2")
                nc.vector.tensor_scalar_mul(o2[:], acc[:, 0:32], rec[:, 0:1])
                nc.sync.dma_start(out=outbuf[t * P:(t + 1) * P, :], in_=o2[:])
    nc.compile()
    return nc


# ----------------------------------------------------------------------------
# entry point
# ----------------------------------------------------------------------------

_CACHE = {}


def kernel(inputs, src, dst, W1, al1, ar1, b1, W2, al2, ar2, b2):
    inputs = np.asarray(inputs, np.float32)
    src = np.asarray(src, np.int32)
    dst = np.asarray(dst, np.int32)
    W1 = np.asarray(W1, np.float32)
    W2 = np.asarray(W2, np.float32)
    al1 = np.asarray(al1, np.float32)
    ar1 = np.asarray(ar1, np.float32)
    al2 = np.asarray(al2, np.float32)
    ar2 = np.asarray(ar2, np.float32)

    prep = _prep(src, dst)
    T, TS, idxs = prep["T"], prep["TS"], prep["idxs"]
    newid, xt_order = prep["newid"], prep["xt_order"]

    key = ("l", tuple(T.tolist()))
    if key not in _CACHE:
        _CACHE[key] = (_build_launch1(T), _build_launch2(T))
    nc1, nc2 = _CACHE[key]

    # host-side tensors
    wl1 = np.einsum("kd,hd->kh", W1.reshape(128, H1, D1).reshape(128, -1)
                    .reshape(128, H1, D1), al1)            # [128, 8]
    wr1 = np.einsum("khd,hd->kh", W1.reshape(128, H1, D1), ar1)
    wl1 = np.einsum("khd,hd->kh", W1.reshape(128, H1, D1), al1)
    w1aug = np.concatenate([W1, wl1, wr1], axis=1).astype(np.float32)
    wl2 = np.einsum("khd,hd->kh", W2.reshape(256, 1, 32), al2)  # [256, 1]
    wr2 = np.einsum("khd,hd->kh", W2.reshape(256, 1, 32), ar2)
    w2aug = np.concatenate([W2, wl2, wr2], axis=1).astype(np.float32)

    x_perm = np.zeros((NPAD, 128), np.float32)
    x_perm[newid] = inputs
    identity = np.eye(P, dtype=np.float32)
    sent = np.zeros((1, ROW1), np.float32)
    sent[0, 256:264] = SENT_EL

    in_maps1 = []
    for c in range(NCORES):
        xt_c = np.ascontiguousarray(
            x_perm.reshape(GBLOCKS, P, 128)[xt_order[c]].transpose(0, 2, 1)
        )
        in_maps1.append({
            "xt": xt_c,
            "w1aug": w1aug,
            "w2aug": w2aug,
            "identin": identity,
            "sentin": sent,
            "idxin": np.ascontiguousarray(idxs[c]),
        })
    res1 = run_bass_kernel_spmd(nc1, in_maps1, core_ids=list(range(NCORES)))

    # assemble layer-2 table per core
    f2_by_newid = np.concatenate(
        [res1.results[c]["f2out"] for c in range(NCORES)], axis=0
    )  # [NPAD, 34]
    in_maps2 = []
    for c in range(NCORES):
        tab2 = np.zeros((NTAB, ROW2), np.float32)
        rows = _new_row(prep["blockpos"][c] * P)  # row of each global block
        # scatter: block g of NPAD occupies rows rows[g]..rows[g]+127
        src_rows = f2_by_newid.reshape(GBLOCKS, P, ROW2)
        for g in range(GBLOCKS):
            tab2[rows[g]:rows[g] + P] = src_rows[g]
        tab2[SENT_A, 32] = SENT_EL
        tab2[SENT_B, 32] = SENT_EL
        # er2 per (j, tile): own nodes are rows 0..NPC
        er2 = np.ascontiguousarray(
            tab2[:NPC, 33].reshape(TILES, P).T
        )
        in_maps2.append({
            "table2": tab2,
            "idxin": np.ascontiguousarray(idxs[c]),
            "er2in": er2,
            "identin": identity,
        })
    res2 = run_bass_kernel_spmd(nc2, in_maps2, core_ids=list(range(NCORES)))

    out_by_newid = np.concatenate(
        [res2.results[c]["outbuf"] for c in range(NCORES)], axis=0
    )  # [NPAD, 32]
    return np.ascontiguousarray(out_by_newid[newid]).astype(np.float32)


# revision 4
# speedup vs baseline: 1.5151x; 1.5151x over previous
"""2-layer GAT on 8 Trainium2 NeuronCores (Bass/Tile).

Sharding: nodes sorted by in-degree, snake-dealt across 8 cores (6250 ->
padded 6272 per core), tiled 128/tile (49 tiles/core); partition j of tile t
owns one dst node, its incoming edges occupy slots (chunk c, partition j).
Per-core HBM node table row = [f(256)|el(8)] f32 from the projection matmul
x @ [W1|W1.al1|W1.ar1]; per-edge rows fetched by indirect-DMA gather (128
rows/chunk). alpha = exp(leaky_relu(el[src]+er[dst])) (no max-subtraction:
logits are small, softmax is shift-invariant); alpha overwrites the el
column so one identity-matmul per chunk accumulates [sum(alpha*f)|sum(alpha)]
in PSUM; divide, ELU. Layer-2 projection h1 @ [W2|wl2|wr2] per tile; host
assembles the full 34-float-row layer-2 table for launch 2 (same grids).
Padding slots point at a sentinel row (f=0, el=-300 -> alpha ~ 0).
"""
import sys

sys.path.insert(0, "/opt/trn_rl_repo")

import numpy as np

import concourse.bass as bass
import concourse.bacc as bacc
import concourse.tile as tile
from concourse import mybir
from concourse.bass_utils import run_bass_kernel_spmd

N = 50000
E = 800000
P = 128
NCORES = 8
TILES = 49                       # tiles per core
NPC = TILES * P                  # 6272 nodes per core
NPAD = NCORES * NPC              # 50176
GBLOCKS = NPAD // P              # 392 projection blocks
SPLIT_ROW = 25088                # sentinel A position
NTAB = NPAD + 2                  # 50178 table rows (two sentinels)
SENT_A = SPLIT_ROW
SENT_B = NTAB - 1
ROW1 = 264                       # [f 256 | el 8]
ROW2 = 34                        # [f2 32 | el2 1 | er2 1]
H1, D1 = 8, 32
NEG_SLOPE = 0.2
SENT_EL = -300.0
F32 = mybir.dt.float32
I32 = mybir.dt.int32


def _new_row(r):
    return r + (r >= SPLIT_ROW)


def _ap(t, off, dims):
    s = t[:] if not isinstance(t, bass.AP) else t
    return bass.AP(tensor=s.tensor, offset=s.offset + off, ap=[s.ap[0]] + dims)


# ----------------------------------------------------------------------------
# host preprocessing
# ----------------------------------------------------------------------------

def _prep(src, dst):
    deg = np.bincount(dst, minlength=N)
    order = np.argsort(-deg, kind="stable")
    pat = np.concatenate([np.arange(NCORES), np.arange(NCORES - 1, -1, -1)])
    core_of_pos = pat[np.arange(N) % (2 * NCORES)]
    newid = np.empty(N, np.int64)
    for c in range(NCORES):
        nodes_c = order[core_of_pos == c]
        newid[nodes_c] = c * NPC + np.arange(len(nodes_c))

    nd = newid[dst]
    ns = newid[src]

    o = np.argsort(nd, kind="stable")
    nd_s, ns_s = nd[o], ns[o]
    first = np.searchsorted(nd_s, np.arange(NPAD), side="left")
    k_s = np.arange(E) - first[nd_s]

    degn = np.bincount(nd, minlength=NPAD).reshape(NCORES, TILES, P)
    T = degn.max(axis=(0, 2)).clip(min=1).astype(np.int64)   # [TILES]
    offs = np.concatenate([[0], np.cumsum(T)])
    TS = int(offs[-1])

    # per-core block order: own 49 blocks first, then the rest
    blockpos = np.empty((NCORES, GBLOCKS), np.int64)
    xt_order = np.empty((NCORES, GBLOCKS), np.int64)
    for c in range(NCORES):
        own = np.arange(c * TILES, (c + 1) * TILES)
        rest = np.concatenate(
            [np.arange(0, c * TILES), np.arange((c + 1) * TILES, GBLOCKS)]
        )
        bo = np.concatenate([own, rest])
        xt_order[c] = bo
        blockpos[c][bo] = np.arange(GBLOCKS)

    # gather indices (per-core table rows of edge srcs), [NCORES, P, TS]
    idxs = np.full((NCORES, P, TS), SENT_B, np.int32)
    c_s = nd_s // NPC
    t_s = (nd_s % NPC) // P
    j_s = nd_s % P
    slot_s = offs[t_s] + k_s
    rowpos = blockpos[c_s, ns_s // P] * P + (ns_s % P)
    idxs[c_s, j_s, slot_s] = _new_row(rowpos).astype(np.int32)

    return {"newid": newid, "T": T, "idxs": idxs,
            "xt_order": xt_order, "blockpos": blockpos}


# ----------------------------------------------------------------------------
# launch 1: projection + layer-1 edges + layer-2 projection
# ----------------------------------------------------------------------------

def _build_launch1(T):
    TS = int(T.sum())
    Tmax = int(T.max())
    offs = np.concatenate([[0], np.cumsum(T)])
    nc = bacc.Bacc("TRN2", target_bir_lowering=False, debug=False,
                   num_devices=NCORES)
    xt = nc.dram_tensor("xt", [GBLOCKS, P, P], F32, kind="ExternalInput")
    w1aug = nc.dram_tensor("w1aug", [P, 272], F32, kind="ExternalInput")
    w2aug = nc.dram_tensor("w2aug", [P, 2 * ROW2], F32, kind="ExternalInput")
    identin = nc.dram_tensor("identin", [P, P], F32, kind="ExternalInput")
    sentin = nc.dram_tensor("sentin", [1, ROW1], F32, kind="ExternalInput")
    idxin = nc.dram_tensor("idxin", [P, TS], I32, kind="ExternalInput")
    f2out = nc.dram_tensor("f2out", [NPC, ROW2], F32, kind="ExternalOutput")
    table = nc.dram_tensor("table", [NTAB, ROW1], F32, kind="Internal")

    er_sb = nc.alloc_sbuf_tensor("er_sb", [P, TILES * H1], F32).ap()
    idx_sb = nc.alloc_sbuf_tensor("idx_sb", [P, TS], I32).ap()
    ident_sb = nc.alloc_sbuf_tensor("ident_sb", [P, P], F32).ap()
    w2_sb = nc.alloc_sbuf_tensor("w2_sb", [P, 2 * ROW2], F32).ap()

    # ---- phase 1: projection builds the node table --------------------------
    with tile.TileContext(nc) as tc:
        with (
            tc.tile_pool(name="p1sbuf", bufs=3) as pool,
            tc.tile_pool(name="p1psum", bufs=4, space="PSUM") as psum,
            tc.tile_pool(name="p1const", bufs=1) as consts,
        ):
            w1_sb = consts.tile([P, 272], F32)
            nc.sync.dma_start(out=w1_sb[:], in_=w1aug[:])
            nc.sync.dma_start(out=ident_sb, in_=identin[:])
            nc.sync.dma_start(out=w2_sb, in_=w2aug[:])
            nc.sync.dma_start(out=idx_sb, in_=idxin[:])
            sent_sb = consts.tile([1, ROW1], F32)
            nc.sync.dma_start(out=sent_sb[:], in_=sentin[:])
            nc.sync.dma_start(out=table[SENT_A:SENT_A + 1, :], in_=sent_sb[:])
            nc.sync.dma_start(out=table[SENT_B:SENT_B + 1, :], in_=sent_sb[:])
            for b in range(GBLOCKS):
                xtile = pool.tile([P, P], F32, tag="xt")
                nc.sync.dma_start(out=xtile[:], in_=xt[b])
                pp = psum.tile([P, 272], F32, tag="pp")
                nc.tensor.matmul(pp[:], xtile[:], w1_sb[:],
                                 start=True, stop=True)
                fo = pool.tile([P, ROW1], F32, tag="fo")
                nc.scalar.activation(out=fo[:], in_=pp[:, 0:ROW1],
                                     func=mybir.ActivationFunctionType.Copy)
                if b < TILES:
                    nc.vector.tensor_copy(
                        out=er_sb[:, b * H1:(b + 1) * H1], in_=pp[:, 264:272]
                    )
                r0 = int(_new_row(b * P))
                nc.sync.dma_start(out=table[r0:r0 + P, :], in_=fo[:])

    # ---- phase 2: layer-1 edges + layer-2 projection ------------------------
    with tile.TileContext(nc) as tc:
        with (
            tc.tile_pool(name="p2sbuf", bufs=2) as pool,
            tc.tile_pool(name="p2small", bufs=3) as small,
            tc.tile_pool(name="p2psum", bufs=2, space="PSUM") as psum,
            tc.tile_pool(name="p2psumT", bufs=2, space="PSUM") as psumT,
            tc.tile_pool(name="p2psum2", bufs=2, space="PSUM") as psum2,
        ):
            for t in range(TILES):
                Tt = int(T[t])
                o0 = int(offs[t])
                g = pool.tile([P, Tmax * ROW1], F32, tag="g")
                gs = g[:]
                gv = gs.rearrange("p (c f) -> p c f", f=ROW1)
                for c in range(Tt):
                    nc.gpsimd.indirect_dma_start(
                        out=gv[:, c, :],
                        out_offset=None,
                        in_=table[:],
                        in_offset=bass.IndirectOffsetOnAxis(
                            ap=idx_sb[:, o0 + c:o0 + c + 1], axis=0
                        ),
                    )
                # logits lt = el[src] + er[dst]   [P, Tt*8]
                lt = small.tile([P, Tmax * H1], F32, tag="lt")
                el_ap = _ap(gs, 256, [[ROW1, Tt], [1, H1]])
                er_ap = _ap(er_sb, t * H1, [[0, Tt], [1, H1]])
                lt_ap = _ap(lt, 0, [[H1, Tt], [1, H1]])
                nc.vector.tensor_tensor(out=lt_ap, in0=el_ap, in1=er_ap,
                                        op=mybir.AluOpType.add)
                # leaky relu: lt = max(lt, 0.2*lt)
                lt2 = small.tile([P, Tmax * H1], F32, tag="lt2")
                nc.vector.tensor_scalar_mul(lt2[:, :Tt * H1],
                                            lt[:, :Tt * H1], NEG_SLOPE)
                nc.vector.tensor_tensor(out=lt[:, :Tt * H1],
                                        in0=lt[:, :Tt * H1],
                                        in1=lt2[:, :Tt * H1],
                                        op=mybir.AluOpType.max)
                # alpha = exp(lt) -> el column of g
                al_ap = _ap(gs, 256, [[ROW1, Tt], [1, H1]])
                nc.scalar.activation(out=al_ap, in_=lt_ap,
                                     func=mybir.ActivationFunctionType.Exp)
                # msg scale: g[:, :, 0:256] *= alpha (broadcast over d)
                f_ap = _ap(gs, 0, [[ROW1, Tt], [32, H1], [1, 32]])
                ab_ap = _ap(gs, 256, [[ROW1, Tt], [1, H1], [0, 32]])
                nc.vector.tensor_tensor(out=f_ap, in0=f_ap, in1=ab_ap,
                                        op=mybir.AluOpType.mult)
                # aggregate: acc = [sum alpha*f | sum alpha]
                acc = psum.tile([P, ROW1], F32, tag="acc")
                for c in range(Tt):
                    nc.tensor.matmul(acc[:], ident_sb, gv[:, c, :],
                                     start=(c == 0), stop=(c == Tt - 1))
                # h1 = elu(acc[:, :256] / denom)   (b1 == 0)
                rec = small.tile([P, H1], F32, tag="rec")
                nc.vector.reciprocal(rec[:], acc[:, 256:ROW1])
                h1 = pool.tile([P, 256], F32, tag="h1")
                acc_f = _ap(acc, 0, [[32, H1], [1, 32]])
                rb_ap = _ap(rec, 0, [[1, H1], [0, 32]])
                h1_ap = _ap(h1, 0, [[32, H1], [1, 32]])
                nc.vector.tensor_tensor(out=h1_ap, in0=acc_f, in1=rb_ap,
                                        op=mybir.AluOpType.mult)
                # ELU: h1 = max(h1, exp(min(h1,0)) - 1)
                e1 = pool.tile([P, 256], F32, tag="e1")
                nc.vector.tensor_scalar_min(e1[:], h1[:], 0.0)
                nc.scalar.activation(out=e1[:], in_=e1[:],
                                     func=mybir.ActivationFunctionType.Exp)
                nc.vector.tensor_scalar_add(e1[:], e1[:], -1.0)
                nc.vector.tensor_tensor(out=h1[:], in0=h1[:], in1=e1[:],
                                        op=mybir.AluOpType.max)
                # layer-2 projection: f2 = h1 @ w2aug
                f2p = psum2.tile([P, ROW2], F32, tag="f2p")
                for k in range(2):
                    tp = psumT.tile([P, P], F32, tag="tp")
                    nc.tensor.transpose(out=tp[:],
                                        in_=h1[:, k * P:(k + 1) * P],
                                        identity=ident_sb)
                    h1t = small.tile([P, P], F32, tag="h1t")
                    nc.vector.tensor_copy(out=h1t[:], in_=tp[:])
                    nc.tensor.matmul(f2p[:], h1t[:],
                                     w2_sb[:, k * ROW2:(k + 1) * ROW2],
                                     start=(k == 0), stop=(k == 1))
                f2s = small.tile([P, ROW2], F32, tag="f2s")
                nc.scalar.activation(out=f2s[:], in_=f2p[:],
                                     func=mybir.ActivationFunctionType.Copy)
                nc.sync.dma_start(out=f2out[t * P:(t + 1) * P, :], in_=f2s[:])
    nc.compile()
    return nc


# ----------------------------------------------------------------------------
# launch 2: layer-2 edge aggregation
# ----------------------------------------------------------------------------

def _build_launch2(T):
    TS = int(T.sum())
    Tmax = int(T.max())
    offs = np.concatenate([[0], np.cumsum(T)])
    nc = bacc.Bacc("TRN2", target_bir_lowering=False, debug=False,
                   num_devices=NCORES)
    table2 = nc.dram_tensor("table2", [NTAB, ROW2], F32, kind="ExternalInput")
    idxin = nc.dram_tensor("idxin", [P, TS], I32, kind="ExternalInput")
    er2in = nc.dram_tensor("er2in", [P, TILES], F32, kind="ExternalInput")
    identin = nc.dram_tensor("identin", [P, P], F32, kind="ExternalInput")
    outbuf = nc.dram_tensor("outbuf", [NPC, 32], F32, kind="ExternalOutput")

    with tile.TileContext(nc) as tc:
        with (
            tc.tile_pool(name="l2sbuf", bufs=2) as pool,
            tc.tile_pool(name="l2small", bufs=3) as small,
            tc.tile_pool(name="l2psum", bufs=3, space="PSUM") as psum,
            tc.tile_pool(name="l2const", bufs=1) as consts,
        ):
            ident_sb = consts.tile([P, P], F32)
            nc.sync.dma_start(out=ident_sb[:], in_=identin[:])
            idx_sb = consts.tile([P, TS], I32)
            nc.sync.dma_start(out=idx_sb[:], in_=idxin[:])
            er2_sb = consts.tile([P, TILES], F32)
            nc.sync.dma_start(out=er2_sb[:], in_=er2in[:])
            for t in range(TILES):
                Tt = int(T[t])
                o0 = int(offs[t])
                g = pool.tile([P, Tmax * ROW2], F32, tag="g")
                gs = g[:]
                gv = gs.rearrange("p (c f) -> p c f", f=ROW2)
                for c in range(Tt):
                    nc.gpsimd.indirect_dma_start(
                        out=gv[:, c, :],
                        out_offset=None,
                        in_=table2[:],
                        in_offset=bass.IndirectOffsetOnAxis(
                            ap=idx_sb[:, o0 + c:o0 + c + 1], axis=0
                        ),
                    )
                lt = small.tile([P, Tmax], F32, tag="lt")
                el_ap = _ap(gs, 32, [[ROW2, Tt]])
                er_ap = _ap(er2_sb, t, [[0, Tt]])
                nc.vector.tensor_tensor(out=lt[:, :Tt], in0=el_ap, in1=er_ap,
                                        op=mybir.AluOpType.add)
                lt2 = small.tile([P, Tmax], F32, tag="lt2")
                nc.vector.tensor_scalar_mul(lt2[:, :Tt], lt[:, :Tt], NEG_SLOPE)
                nc.vector.tensor_tensor(out=lt[:, :Tt], in0=lt[:, :Tt],
                                        in1=lt2[:, :Tt],
                                        op=mybir.AluOpType.max)
                al_ap = _ap(gs, 32, [[ROW2, Tt]])
                nc.scalar.activation(out=al_ap, in_=lt[:, :Tt],
                                     func=mybir.ActivationFunctionType.Exp)
                f_ap = _ap(gs, 0, [[ROW2, Tt], [1, 32]])
                ab_ap = _ap(gs, 32, [[ROW2, Tt], [0, 32]])
                nc.vector.tensor_tensor(out=f_ap, in0=f_ap, in1=ab_ap,
                                        op=mybir.AluOpType.mult)
                acc = psum.tile([P, 33], F32, tag="acc")
                for c in range(Tt):
                    nc.tensor.matmul(acc[:], ident_sb[:], gv[:, c, 0:33],
                                     start=(c == 0), stop=(c == Tt - 1))
                rec = small.tile([P, 1], F32, tag="rec")
                nc.vector.reciprocal(rec[:], acc[:, 32:33])
                o2 = small.tile([P, 32], F32, tag="o2")
                nc.vector.tensor_scalar_mul(o2[:], acc[:, 0:32], rec[:, 0:1])
                nc.sync.dma_start(out=outbuf[t * P:(t + 1) * P, :], in_=o2[:])
    nc.compile()
    return nc


# ----------------------------------------------------------------------------
# entry point
# ----------------------------------------------------------------------------

_CACHE = {}
PROFILE = False
LAST_EXEC_NS = []


def _run(nc, in_maps, tag):
    if PROFILE:
        import tempfile
        res = run_bass_kernel_spmd(
            nc, in_maps, core_ids=list(range(NCORES)), trace=True,
            tmpdir=tempfile.mkdtemp(prefix=f"gat_{tag}_"),
        )
        LAST_EXEC_NS.append((tag, res.exec_time_ns))
        return res
    return run_bass_kernel_spmd(nc, in_maps, core_ids=list(range(NCORES)))


def kernel(inputs, src, dst, W1, al1, ar1, b1, W2, al2, ar2, b2):
    inputs = np.asarray(inputs, np.float32)
    src = np.asarray(src).astype(np.int64)
    dst = np.asarray(dst).astype(np.int64)
    W1 = np.asarray(W1, np.float32)
    W2 = np.asarray(W2, np.float32)
    al1 = np.asarray(al1, np.float32)
    ar1 = np.asarray(ar1, np.float32)
    al2 = np.asarray(al2, np.float32)
    ar2 = np.asarray(ar2, np.float32)

    prep = _prep(src, dst)
    T, idxs = prep["T"], prep["idxs"]
    newid, xt_order = prep["newid"], prep["xt_order"]

    key = tuple(T.tolist())
    if key not in _CACHE:
        _CACHE[key] = (_build_launch1(T), _build_launch2(T))
    nc1, nc2 = _CACHE[key]

    wl1 = np.einsum("khd,hd->kh", W1.reshape(128, H1, D1), al1)
    wr1 = np.einsum("khd,hd->kh", W1.reshape(128, H1, D1), ar1)
    w1aug = np.concatenate([W1, wl1, wr1], axis=1).astype(np.float32)
    wl2 = np.einsum("khd,hd->kh", W2.reshape(256, 1, 32), al2)
    wr2 = np.einsum("khd,hd->kh", W2.reshape(256, 1, 32), ar2)
    w2a = np.concatenate([W2, wl2, wr2], axis=1).astype(np.float32)  # [256,34]
    w2aug = np.concatenate([w2a[:P], w2a[P:]], axis=1)               # [128,68]

    x_perm = np.zeros((NPAD, 128), np.float32)
    x_perm[newid] = inputs
    identity = np.eye(P, dtype=np.float32)
    sent = np.zeros((1, ROW1), np.float32)
    sent[0, 256:264] = SENT_EL

    in_maps1 = []
    for c in range(NCORES):
        xt_c = np.ascontiguousarray(
            x_perm.reshape(GBLOCKS, P, 128)[xt_order[c]].transpose(0, 2, 1)
        )
        in_maps1.append({
            "xt": xt_c, "w1aug": w1aug, "w2aug": w2aug,
            "identin": identity, "sentin": sent,
            "idxin": np.ascontiguousarray(idxs[c]),
        })
    res1 = _run(nc1, in_maps1, "l1")

    f2_by_newid = np.concatenate(
        [res1.results[c]["f2out"] for c in range(NCORES)], axis=0
    ).reshape(GBLOCKS, P, ROW2)
    in_maps2 = []
    for c in range(NCORES):
        tab2 = np.zeros((NTAB, ROW2), np.float32)
        rows = _new_row(prep["blockpos"][c] * P)
        for gblk in range(GBLOCKS):
            tab2[rows[gblk]:rows[gblk] + P] = f2_by_newid[gblk]
        tab2[SENT_A, 32] = SENT_EL
        tab2[SENT_B, 32] = SENT_EL
        er2 = np.ascontiguousarray(tab2[:NPC, 33].reshape(TILES, P).T)
        in_maps2.append({
            "table2": tab2,
            "idxin": np.ascontiguousarray(idxs[c]),
            "er2in": er2,
            "identin": identity,
        })
    res2 = _run(nc2, in_maps2, "l2")

    out_by_newid = np.concatenate(
        [res2.results[c]["outbuf"] for c in range(NCORES)], axis=0
    )
    return np.ascontiguousarray(out_by_newid[newid]).astype(np.float32)
